# revision 62
# baseline (speedup 1.0000x reference)
"""VQ codebook kernel (nn_KW_CascadedBranch) for 8 Trainium2 NeuronCores.

Reference computation:
    kw   = audio_feat @ proj_w + proj_b                  [B,N,512]
    cos  = normalize(kw) @ normalize(token_embedding).T  [B,N,V]
    p    = softmax(cos / 0.1)
    out  = p @ token_embedding                           [B,N,512]

Strategy: tensor-parallel over the vocab dim V=49408. Each core owns a
6176-row shard (padded to 6400 = 50*128) and computes partial (p @ emb)
plus partial softmax denominators for all B*N=2048 keyword slots; the
host combines the 8 partials.

The two big GEMMs run as fp8(e4m3) DoubleRow matmuls (0.5 cycles/row,
4x over fp32r). Precision: the keyword-side quantization error is
coherent across the vocab (it biases every logit of a slot the same
way), so kwn is split hi+lo fp8 (2-term GEMM1); the emb-side and
p-side errors average out incoherently over 49k vocab entries, so emb
and p stay 1-term fp8 (measured end-to-end max-rel ~8e-3 vs 2e-2 gate).

    GEMM1 scores[v,m] = et^T (kh + kl): 4 DR matmuls per v-tile
    p8 = exp(scale_v * scores + mask)      (fp8 out, ACT)
    GEMM2 out[e,m] += emb8[v,e]^T p8, denominator via a DR ones-matmul

The projection runs transposed (kwT[e,m] = pw^T @ audio^T, bf16) so no
PE transposes are needed; proj_b folds into the PSUM->SBUF copy as a
per-partition tensor_scalar add. Keyword norms reduce over partitions
via a ones-matmul + rank-1 broadcast matmul and a short sqrt/recip
chain (DVE reciprocal is accurate enough without a Newton step).
Emb-shard norms square-reduce the natural-layout tiles on ACT
(Square+accum_out) and DVE (scalar_tensor_tensor+accum_out), emitted
just-in-time in groups of 10 v-tiles so they never stall the PE.
All emb-sized tensors are resident in SBUF (~7MB fp8); every input
arrives in a handful of large partition-major DMAs, ordered so m-chunk
0's operands land first. Each next m-chunk's projection/normalize is
emitted mid-pair-loop so PE never waits for it at chunk boundaries.
"""

import numpy as np
import ml_dtypes

import concourse.bass as bass
import concourse.mybir as mybir
from concourse import tile
from concourse.bass_utils import run_bass_kernel_spmd

F32 = mybir.dt.float32
F32R = mybir.dt.float32r
BF16 = mybir.dt.bfloat16
F8 = mybir.dt.float8e4
F8NP = ml_dtypes.float8_e4m3
BF16NP = ml_dtypes.bfloat16
AF = mybir.ActivationFunctionType
OP = mybir.AluOpType
DRMODE = mybir.MatmulPerfMode.DoubleRow

N_CORES = 8
B, N, D, E, V = 256, 8, 768, 512, 49408
M = B * N                      # 2048 keyword slots
DT = D // 128                  # 6 d-chunks
VS = V // N_CORES              # 6176 real vocab rows per core
VT = 50                        # v-tiles of 128 per core (6400 rows, 224 pad)
VP = VT * 128
NPAIR = VT // 2                # 25 DoubleRow v-tile pairs
# staggered m-chunk widths: a narrow first chunk gets real work going
# ~15us earlier (its projection/normalize chain is 4x shorter), the rest
# use full 512-wide PSUM accumulators
MCS = [512, 512, 512, 512]
MCO = [0, 512, 1024, 1536]  # offsets (cumsum)
MC = 512                       # max m-chunk width
NMC = len(MCS)
EC = E // 128                  # 4 e-chunks
S_KW = 256.0                   # kwn fp8 pre-scale
S_EMB = 512.0                  # emb fp8 pre-scale
EXP_SCALE_C = 10.0 / S_KW      # folded into the per-v exp scale
NEG_BIG = -1.0e30


def _split_multiwait_ctrl(nc, max_waits: int = 1) -> int:
    """This container's walrus rejects instructions carrying more than one
    semaphore wait (CTRL and S3_LW encodings alike). Hoist overflow waits
    onto same-engine NoOps inserted immediately before the offender."""
    n_split = 0
    for fn in nc.m.functions:
        for bb in fn.blocks:
            rebuilt, changed = [], False
            for ins in bb.instructions:
                si = ins.sync_info
                if (
                    si is not None
                    and si.on_wait
                    and len(si.on_wait) > max_waits
                ):
                    waits = list(si.on_wait)
                    head, tail = waits[:-max_waits], waits[-max_waits:]
                    for i in range(0, len(head), max_waits):
                        nop = mybir.InstNoOp(name=f"{ins.name}-ws{i}", ins=[], outs=[])
                        nop.engine = ins.engine
                        nop.sync_info = mybir.SyncInfo(
                            on_wait=head[i:i + max_waits], on_update=[]
                        )
                        rebuilt.append(nop)
                    ins.sync_info = mybir.SyncInfo(
                        on_wait=tail, on_update=list(si.on_update or [])
                    )
                    changed = True
                    n_split += 1
                rebuilt.append(ins)
            if changed:
                bb.instructions = rebuilt
    return n_split


def build_program():
    nc = bass.Bass(target_bir_lowering=False)

    # partition-major host layouts so each tensor arrives in 1-4 large DMAs
    audio_r = nc.dram_tensor("audio_r", [128, DT, M], BF16, kind="ExternalInput")
    pb_r = nc.dram_tensor("pb_r", [128, EC], F32, kind="ExternalInput")
    pw_r = nc.dram_tensor("pw_r", [128, DT, E], BF16, kind="ExternalInput")
    et4 = nc.dram_tensor("et4", [128, 2, 2, VP], F8, kind="ExternalInput")
    en4 = nc.dram_tensor("en4", [128, NPAIR, 2, E], F8, kind="ExternalInput")
    mask_b = nc.dram_tensor("mask_b", [128, VT], F32, kind="ExternalInput")

    out_pe = nc.dram_tensor("out_pe", [E, M], F32, kind="ExternalOutput")
    out_d = nc.dram_tensor("out_d", [1, M], F32, kind="ExternalOutput")

    with tile.TileContext(nc) as tc:
        with (
            tc.tile_pool(name="res", bufs=1) as res,
            tc.tile_pool(name="atp", bufs=2) as atp,
            tc.tile_pool(name="sqd", bufs=2) as sqd,
            tc.tile_pool(name="kwp", bufs=1) as kwp,
            tc.tile_pool(name="qp", bufs=3) as qp,
            tc.tile_pool(name="op", bufs=2) as op,
            tc.tile_pool(name="sc_ps", bufs=2, space="PSUM") as sc_ps,
            tc.tile_pool(name="acc_ps", bufs=4, space="PSUM") as acc_ps,
            tc.tile_pool(name="d_ps", bufs=1, space="PSUM") as d_ps,
        ):
            # ---- resident tiles + DMA (emission order = SP issue order) ----
            # JIT priority: mc0's inputs first (audio0, pw, then et/en pieces
            # interleaved in consumption order), audio for mc1-3 last
            at_tiles = {
                mc: atp.tile([128, DT, MCS[mc]], BF16, tag=f"at{mc}", name=f"at{mc}")
                for mc in range(NMC)
            }
            nc.sync.dma_start(at_tiles[0][:, 0:3, :], audio_r[:, 0:3, 0:MCS[0]])
            nc.sync.dma_start(at_tiles[0][:, 3:DT, :], audio_r[:, 3:DT, 0:MCS[0]])
            pw_sb = res.tile([128, DT, E], BF16, tag="pw", name="pw_sb")
            nc.sync.dma_start(pw_sb[:, 0:3, :], pw_r[:, 0:3, :])
            nc.sync.dma_start(pw_sb[:, 3:DT, :], pw_r[:, 3:DT, :])
            mask_sb = res.tile([128, VT], F32, tag="mask", name="mask_sb")
            nc.sync.dma_start(mask_sb[:], mask_b[:])
            pb_sb = res.tile([128, EC], F32, tag="pb", name="pb_sb")
            nc.sync.dma_start(pb_sb[:], pb_r[:])

            et_sb = res.tile([128, 2, 2, VP], F8, tag="et", name="et_sb")
            en_sb = res.tile([128, NPAIR, 2, E], F8, tag="en", name="en_sb")
            EPC = VP // 4  # 1600-col et pieces, v-ordered
            en_cuts = [0, 7, 13, 19, NPAIR]
            for pc in range(4):
                sl = slice(pc * EPC, (pc + 1) * EPC)
                nc.sync.dma_start(et_sb[:, :, :, sl], et4[:, :, :, sl])
                tsl = slice(en_cuts[pc], en_cuts[pc + 1])
                nc.sync.dma_start(en_sb[:, tsl, :, :], en4[:, tsl, :, :])
            for mc in range(1, NMC):
                nc.sync.dma_start(
                    at_tiles[mc][:],
                    audio_r[:, :, MCO[mc]:MCO[mc] + MCS[mc]],
                )

            ensq = res.tile([128, VT], F32, tag="ensq", name="ensq")
            scale_e = res.tile([128, VT], F32, tag="scale_e", name="scale_e")
            onesf = res.tile([128, 128], F32, tag="onesf", name="onesf")
            nc.vector.memset(onesf[:], 1.0)
            ones2 = res.tile([128, 32], F8, tag="ones2", name="ones2")
            nc.vector.tensor_copy(ones2[:], onesf[:, 0:32])
            ones_col = res.tile([128, 1], F32R, tag="ones_col", name="ones_col")
            nc.scalar.copy(ones_col[:], onesf[:, 0:1])
            ones_row = res.tile([1, 128], F32R, tag="ones_row", name="ones_row")
            nc.scalar.copy(ones_row[:], onesf[0:1, :])

            khT = [[res.tile([128, 2, MCS[mc]], F8, tag=f"khT{jj}_{mc}", name=f"khT{jj}_{mc}")
                    for mc in range(NMC)] for jj in range(2)]
            klT = [[res.tile([128, 2, MCS[mc]], F8, tag=f"klT{jj}_{mc}", name=f"klT{jj}_{mc}")
                    for mc in range(NMC)] for jj in range(2)]

            # ---- keyword projection prologue (transposed: kwT[e, m]) ----
            def prologue(mc, ps_tag="pro"):
                pbufs = 2 if ps_tag == "scores" else 1
                w, off = MCS[mc], MCO[mc]
                ats = at_tiles[mc]
                kwT_sb = []
                sqs = []
                for j in range(EC):
                    kwT_ps = sc_ps.tile([128, w], F32, tag=ps_tag, bufs=pbufs, name=f"kwT{mc}_{j}")
                    for d in range(DT):
                        nc.tensor.matmul(
                            kwT_ps[:], pw_sb[:, d, j * 128:(j + 1) * 128],
                            ats[:, d, 0:w],
                            start=(d == 0), stop=(d == DT - 1),
                        )
                    ksb = kwp.tile([128, MC], F32, tag=f"kwTs{j}", name=f"kwTs{mc}_{j}")
                    nc.vector.tensor_scalar_add(ksb[:, 0:w], kwT_ps[:], pb_sb[:, j:j + 1])
                    kwT_sb.append(ksb)
                    sq = kwp.tile([128, MC], F32, tag=f"sqkw{j}", name=f"sqkw{mc}_{j}")
                    nc.gpsimd.tensor_mul(sq[:, 0:w], ksb[:, 0:w], ksb[:, 0:w])
                    sqs.append(sq)
                sqacc = kwp.tile([128, MC], F32R, tag="sqacc", name=f"sqacc{mc}")
                nc.vector.tensor_add(sqacc[:, 0:w], sqs[0][:, 0:w], sqs[1][:, 0:w])
                nc.vector.tensor_add(sqacc[:, 0:w], sqacc[:, 0:w].bitcast(F32), sqs[2][:, 0:w])
                nc.vector.tensor_add(sqacc[:, 0:w], sqacc[:, 0:w].bitcast(F32), sqs[3][:, 0:w])
                # partition-dim reduce via ones matmul -> [1, w], then chain
                nsq_ps = sc_ps.tile([128, w], F32, tag=ps_tag, bufs=pbufs, name=f"nsq_ps{mc}")
                nc.tensor.matmul(nsq_ps[0:1, :], ones_col[:], sqacc[:, 0:w])
                # rs = S_KW * rsqrt(nsq): ACT sqrt straight from PSUM, then
                # DVE reciprocal (scale folded into the recip via tensor_scalar)
                s_k = kwp.tile([1, MC], F32, tag="s_k", name=f"s_k{mc}")
                s_k = s_k[0:1, 0:w]
                nc.scalar.activation(s_k, nsq_ps[0:1, :], AF.Sqrt)
                r0 = kwp.tile([1, MC], F32, tag="r0_k", name=f"r0_k{mc}")
                r0 = r0[0:1, 0:w]
                nc.vector.reciprocal(r0, s_k)
                rs_row = kwp.tile([1, MC], F32R, tag="rs_row", name=f"rs_row{mc}")
                nc.vector.tensor_scalar_mul(rs_row[0:1, 0:w], r0, S_KW)
                # broadcast rs to all partitions via rank-1 matmul
                rs_ps = sc_ps.tile([128, w], F32, tag=ps_tag, bufs=pbufs, name=f"rs_ps{mc}")
                nc.tensor.matmul(rs_ps[:], ones_row[:], rs_row[0:1, 0:w])
                rs = kwp.tile([128, MC], F32, tag="rs", name=f"rs{mc}")
                nc.vector.tensor_copy(rs[:, 0:w], rs_ps[:])
                for j in range(EC):
                    jj, i = j // 2, j % 2
                    tmp = kwp.tile([128, MC], F32, tag="tmpk", bufs=2, name=f"tmpk{mc}_{j}")
                    nc.vector.tensor_mul(tmp[:, 0:w], kwT_sb[j][:, 0:w], rs[:, 0:w])
                    nc.vector.tensor_copy(khT[jj][mc][:, i, 0:w], tmp[:, 0:w])
                    nc.vector.tensor_sub(
                        klT[jj][mc][:, i, 0:w], tmp[:, 0:w], khT[jj][mc][:, i, 0:w]
                    )

            prologue(0, ps_tag="scores")

            # ---- emb row norms from the natural-layout tiles ----
            # ensq[:, k] = sum_e en^2, spread across ACT (Square+accum),
            # DVE (scalar_tensor_tensor+accum) and Pool (mul+reduce) so no
            # single engine gates the softmax scale pipeline.
            ENSQ_C = EXP_SCALE_C
            def emit_ensq_group(g):
                for k in range(g * 10, (g + 1) * 10):
                    en_slice = en_sb[:, k // 2, k % 2, :]
                    if g == 0 or k % 2 == 0:
                        dump = sqd.tile([128, E], F32, tag="dumpA", name=f"dumpA{k}")
                        nc.scalar.activation(
                            dump[:], en_slice, AF.Square,
                            accum_out=ensq[:, k:k + 1],
                        )
                    else:
                        dump = sqd.tile([128, E], F32, tag="dumpV", name=f"dumpV{k}")
                        nc.vector.scalar_tensor_tensor(
                            dump[:], en_slice, 1.0, en_slice, OP.mult, OP.mult,
                            accum_out=ensq[:, k:k + 1],
                        )
                # scale_e = (EXP_SCALE_C/8) * rsqrt(ensq/64) for this group
                sl = slice(g * 10, (g + 1) * 10)
                nc.vector.tensor_scalar_add(ensq[:, sl], ensq[:, sl], 1e-24)
                s_e = sqd.tile([128, 16], F32, tag="s_e", name=f"s_e{g}")
                se = s_e[:, 0:10]
                nc.scalar.activation(se, ensq[:, sl], AF.Sqrt)
                r_e = sqd.tile([128, 16], F32, tag="r_e", name=f"r_e{g}")
                re = r_e[:, 0:10]
                nc.vector.reciprocal(re, se)
                nc.vector.tensor_scalar_mul(scale_e[:, sl], re, ENSQ_C)

            emit_ensq_group(0)

            # ---- main loop ----
            def main(mc):
                w, off = MCS[mc], MCO[mc]
                kwacc = [
                    acc_ps.tile([128, w], F32, tag="kwacc", name=f"kwacc{mc}_{j}")
                    for j in range(EC)
                ]
                dacc = d_ps.tile([1, w], F32, tag="dacc", name=f"dacc{mc}")

                def emit_g2(q2, t):
                    for j in range(EC):
                        nc.tensor.matmul(
                            kwacc[j][:], en_sb[:, t, :, j * 128:(j + 1) * 128], q2[:],
                            start=(t == 0), stop=(t == NPAIR - 1), perf_mode=DRMODE,
                        )
                    ones2_3d = ones2[:].rearrange("p (a b) -> p a b", a=2)
                    nc.tensor.matmul(
                        dacc[:], ones2_3d[:, :, 0:1], q2[:],
                        start=(t == 0), stop=(t == NPAIR - 1), perf_mode=DRMODE,
                    )

                prev = None
                for t in range(NPAIR):
                    if mc == 0 and t in (3, 8, 13, 18):
                        # JIT norm groups: emitted two pairs ahead of use so
                        # their DVE chain never head-of-line-blocks the
                        # prologue's normalize ops
                        emit_ensq_group((t + 2) // 5)
                    if t == 15 and mc < NMC - 1:
                        # overlap the next m-chunk's projection + normalize
                        # with the tail of this m-chunk's pair loop
                        prologue(mc + 1)
                    q2 = qp.tile([128, 2, w], F8, tag="q2", name=f"q2_{mc}_{t}")
                    for half in range(2):
                        k = 2 * t + half
                        scores = sc_ps.tile([128, w], F32, tag="scores", name=f"sc{mc}_{k}")
                        mm = 0
                        for tiles in (khT, klT):
                            for jj in range(2):
                                nc.tensor.matmul(
                                    scores[:],
                                    et_sb[:, jj, :, k * 128:(k + 1) * 128],
                                    tiles[jj][mc][:],
                                    start=(mm == 0), stop=(mm == 3), perf_mode=DRMODE,
                                )
                                mm += 1
                        nc.scalar.activation(
                            q2[:, half, :], scores[:], AF.Exp,
                            bias=mask_sb[:, k:k + 1],
                            scale=scale_e[:, k:k + 1],
                        )
                    if prev is not None:
                        emit_g2(*prev)
                    prev = (q2, t)
                emit_g2(*prev)

                # flush: copies split ACT/DVE, per-e-chunk DMAs pipeline the
                # tail instead of waiting for all four copies
                osb = op.tile([128, EC, MC], F32, tag="osb", name=f"osb{mc}")
                for j in range(EC):
                    if mc == NMC - 1 and j % 2 == 0:
                        nc.scalar.copy(osb[:, j, 0:w], kwacc[j][:])
                    else:
                        nc.vector.tensor_copy(osb[:, j, 0:w], kwacc[j][:])
                    nc.sync.dma_start(
                        out_pe[j * 128:(j + 1) * 128, off:off + w],
                        osb[:, j, 0:w],
                    )
                dsb = op.tile([1, MC], F32, tag="dsb", name=f"dsb{mc}")
                nc.vector.tensor_copy(dsb[0:1, 0:w], dacc[:])
                nc.sync.dma_start(out_d[:, off:off + w], dsb[0:1, 0:w])

            for mc in range(NMC):
                main(mc)
    return nc


_CACHED = {}


def _get_program():
    if "nc" not in _CACHED:
        nc = build_program()
        _split_multiwait_ctrl(nc)
        _CACHED["nc"] = nc
    return _CACHED["nc"]


def _prep_in_maps(audio_feat, proj_w, proj_b, token_embedding):
    audio = np.asarray(audio_feat, np.float32).reshape(M, D)
    audio_r = np.ascontiguousarray(
        audio.T.reshape(DT, 128, M).transpose(1, 0, 2)
    ).astype(BF16NP)
    pw_r = np.ascontiguousarray(
        np.asarray(proj_w, np.float32).reshape(DT, 128, E).transpose(1, 0, 2)
    ).astype(BF16NP)
    pb_r = np.ascontiguousarray(
        np.asarray(proj_b, np.float32).reshape(EC, 128).T
    )

    mask = np.zeros((128, VT), np.float32)
    nreal_last = VS - (VT - 2) * 128          # 32 real rows in v-tile 48
    mask[nreal_last:, VT - 2] = NEG_BIG
    mask[:, VT - 1] = NEG_BIG

    emb = np.asarray(token_embedding, np.float32)
    in_maps = []
    for c in range(N_CORES):
        shard = np.zeros((VP, E), np.float32)
        shard[:VS] = emb[c * VS:(c + 1) * VS]
        eh8 = (shard * S_EMB).astype(F8NP)                       # [VP, E]
        etT = np.ascontiguousarray(eh8.T)                        # [E, VP]
        et = np.ascontiguousarray(
            etT.reshape(2, 2, 128, VP).transpose(2, 0, 1, 3)    # [128,2,2,VP]
        )
        en = np.ascontiguousarray(
            eh8.reshape(NPAIR, 2, 128, E).transpose(2, 0, 1, 3)  # [128,25,2,E]
        )
        in_maps.append({
            "audio_r": audio_r,
            "pw_r": pw_r,
            "pb_r": pb_r,
            "et4": et,
            "en4": en,
            "mask_b": mask,
        })
    return in_maps


def kernel(audio_feat, proj_w, proj_b, token_embedding, _trace=False):
    nc = _get_program()
    in_maps = _prep_in_maps(audio_feat, proj_w, proj_b, token_embedding)
    res = run_bass_kernel_spmd(
        nc, in_maps, core_ids=list(range(N_CORES)), trace=_trace
    )
    pe = np.zeros((E, M), np.float64)
    dn = np.zeros((1, M), np.float64)
    for c in range(N_CORES):
        pe += res.results[c]["out_pe"]
        dn += res.results[c]["out_d"]
    out = (pe / dn / S_EMB).T.reshape(B, N, E).astype(np.float32)
    if _trace:
        return out, res
    return out
